# revision 1
# baseline (speedup 1.0000x reference)
"""GNN message passing (2-layer, residual) on 8 TRN2 NeuronCores.

Strategy: shard destination nodes across 8 cores (12500 rows each, 98
blocks of 128). Host sorts edges by (dest block, src), pads each block
to T slices of 128 edges. Device gathers neighbor rows by src index
(indirect DMA), scatter-adds them into the dest block via a one-hot
matmul accumulated in PSUM (aggT = G.T @ M), then applies the per-layer
linear/relu. Two launches: layer 0 produces h shards, host concats the
full h (halo exchange), launch 2 does layer 1 + residual + projection.
"""
import os
import sys
import types
import contextlib
import ctypes

import numpy as np

import concourse.bass as bass
import concourse.tile as tile
from concourse import bacc, mybir
from concourse.bass_utils import run_bass_kernel_spmd

N = 100000
E = 640000
D = 128
NC = 8
R = N // NC          # 12500 rows per core
NB = (R + 127) // 128  # 98 blocks; last block has 84 rows
P = 128

PROFILE = bool(int(os.environ.get("GNN_PROFILE", "0")))
LAST_EXEC_NS = []    # per-launch exec_time_ns when PROFILE


def _install_ntff_shim():
    if "antenv.axon_hooks" in sys.modules:
        return
    mod = types.ModuleType("antenv.axon_hooks")
    mod._hook = None
    mod.set_axon_ntff_profile_hook = lambda h: setattr(mod, "_hook", h)
    mod.get_axon_ntff_profile_hook = lambda: mod._hook
    sys.modules["antenv.axon_hooks"] = mod
    try:
        import antenv
        antenv.axon_hooks = mod
        from trn_agent_boot.trn_boot import _ntff_profile_via_ctypes
        mod.set_axon_ntff_profile_hook(
            _ntff_profile_via_ctypes("/opt/axon/libaxon_pjrt.so"))
    except Exception:
        pass


def _prep_edges(edge_index):
    """Per-core padded slice schedule. Per-block slice count T_b is the max
    over cores (SPMD: one program for all cores). Returns colsT [NC,128,S]
    i32, rlT [NC,128,S] f32 (128.0 = padding sentinel), T_arr [NB], offs
    [NB] (slice start per block)."""
    row = edge_index[0].astype(np.int64)
    col = edge_index[1].astype(np.int64)
    per_core = []
    tmax = np.zeros(NB, dtype=np.int64)
    for k in range(NC):
        m = (row // R) == k
        r_loc = (row[m] - k * R).astype(np.int64)
        c = col[m].astype(np.int32)
        blk = r_loc >> 7
        rl = (r_loc & 127).astype(np.int32)
        order = np.lexsort((c, blk))
        blk, rl, c = blk[order], rl[order], c[order]
        counts = np.bincount(blk, minlength=NB)
        tmax = np.maximum(tmax, (counts + P - 1) // P)
        per_core.append((blk, rl, c, counts))
    T_arr = np.maximum(tmax, 1)
    offs = np.zeros(NB, dtype=np.int64)
    offs[1:] = np.cumsum(T_arr)[:-1]
    S = int(T_arr.sum())
    colsT = np.zeros((NC, P, S), dtype=np.int32)
    rlT = np.full((NC, P, S), 128.0, dtype=np.float32)
    for k in range(NC):
        blk, rl, c, counts = per_core[k]
        starts = np.zeros(NB, dtype=np.int64)
        starts[1:] = np.cumsum(counts)[:-1]
        pos = np.arange(len(blk)) - starts[blk]
        s = offs[blk] + pos // P
        p = pos % P
        colsT[k][p, s] = c
        rlT[k][p, s] = rl.astype(np.float32)
    return colsT, rlT, T_arr, offs


def _build_layer0(T_arr, offs):
    S = int(T_arr.sum())
    nc = bacc.Bacc("TRN2", target_bir_lowering=False, debug=False,
                   num_devices=NC)
    x_d = nc.dram_tensor("x", [N, D], mybir.dt.float32, kind="ExternalInput")
    cols_d = nc.dram_tensor("cols", [P, S], mybir.dt.int32, kind="ExternalInput")
    rl_d = nc.dram_tensor("rl", [P, S], mybir.dt.float32, kind="ExternalInput")
    w0_d = nc.dram_tensor("w0", [D, D], mybir.dt.float32, kind="ExternalInput")
    b0_d = nc.dram_tensor("b0", [1, D], mybir.dt.float32, kind="ExternalInput")
    h_d = nc.dram_tensor("h", [R, D], mybir.dt.float32, kind="ExternalOutput")

    with tile.TileContext(nc) as tc:
        with contextlib.ExitStack() as ctx:
            const = ctx.enter_context(tc.tile_pool(name="const", bufs=1))
            gp = ctx.enter_context(tc.tile_pool(name="gp", bufs=6))
            mp = ctx.enter_context(tc.tile_pool(name="mp", bufs=6))
            sp = ctx.enter_context(tc.tile_pool(name="sp", bufs=3))
            hp = ctx.enter_context(tc.tile_pool(name="hp", bufs=3))
            pa = ctx.enter_context(tc.tile_pool(name="pa", bufs=2, space="PSUM"))
            ph = ctx.enter_context(tc.tile_pool(name="ph", bufs=2, space="PSUM"))

            colsSB = const.tile([P, S], mybir.dt.int32)
            rlSB = const.tile([P, S], mybir.dt.float32)
            nc.sync.dma_start(out=colsSB[:], in_=cols_d[:])
            nc.sync.dma_start(out=rlSB[:], in_=rl_d[:])
            w0SB = const.tile([D, D], mybir.dt.float32)
            b0SB = const.tile([1, D], mybir.dt.float32)
            nc.sync.dma_start(out=w0SB[:], in_=w0_d[:])
            nc.sync.dma_start(out=b0SB[:], in_=b0_d[:])
            ones1 = const.tile([1, P], mybir.dt.float32)
            nc.vector.memset(ones1[:], 1.0)
            iotaI = const.tile([P, P], mybir.dt.int32)
            nc.gpsimd.iota(iotaI[:], pattern=[[1, P]], base=0,
                           channel_multiplier=0)
            iotaF = const.tile([P, P], mybir.dt.float32)
            nc.vector.tensor_copy(iotaF[:], iotaI[:])

            for b in range(NB):
                rows_b = min(P, R - b * P)
                T_b = int(T_arr[b])
                psumA = pa.tile([P, P], mybir.dt.float32, tag="pa")
                for j in range(T_b):
                    s = int(offs[b]) + j
                    gb = gp.tile([P, P], mybir.dt.float32, tag="g")
                    nc.gpsimd.indirect_dma_start(
                        out=gb[:], out_offset=None, in_=x_d[:],
                        in_offset=bass.IndirectOffsetOnAxis(
                            ap=colsSB[:, s:s + 1], axis=0))
                    M = mp.tile([P, P], mybir.dt.float32, tag="m")
                    nc.vector.tensor_scalar(
                        out=M[:], in0=iotaF[:], scalar1=rlSB[:, s:s + 1],
                        scalar2=None, op0=mybir.AluOpType.is_equal)
                    nc.tensor.matmul(psumA[:], lhsT=gb[:], rhs=M[:],
                                     start=(j == 0), stop=(j == T_b - 1))
                sA = sp.tile([P, P], mybir.dt.float32, tag="sa")
                nc.vector.tensor_copy(sA[:], psumA[:])
                psumH = ph.tile([P, P], mybir.dt.float32, tag="phh")
                nc.tensor.matmul(psumH[:], lhsT=sA[:], rhs=w0SB[:],
                                 start=True, stop=False)
                nc.tensor.matmul(psumH[:], lhsT=ones1[:], rhs=b0SB[:],
                                 start=False, stop=True)
                hsb = hp.tile([P, P], mybir.dt.float32, tag="h")
                nc.scalar.activation(hsb[:], psumH[:],
                                     mybir.ActivationFunctionType.Relu)
                nc.sync.dma_start(out=h_d[b * P:b * P + rows_b, :],
                                  in_=hsb[:rows_b, :])
    nc.compile()
    return nc


def _build_layer1(T_arr, offs):
    S = int(T_arr.sum())
    nc = bacc.Bacc("TRN2", target_bir_lowering=False, debug=False,
                   num_devices=NC)
    hf_d = nc.dram_tensor("hf", [N, D], mybir.dt.float32, kind="ExternalInput")
    cols_d = nc.dram_tensor("cols", [P, S], mybir.dt.int32, kind="ExternalInput")
    rl_d = nc.dram_tensor("rl", [P, S], mybir.dt.float32, kind="ExternalInput")
    w1_d = nc.dram_tensor("w1", [D, D], mybir.dt.float32, kind="ExternalInput")
    b1_d = nc.dram_tensor("b1", [P, 1], mybir.dt.float32, kind="ExternalInput")
    wp_d = nc.dram_tensor("wp", [D, D], mybir.dt.float32, kind="ExternalInput")
    bp_d = nc.dram_tensor("bp", [1, D], mybir.dt.float32, kind="ExternalInput")
    o_d = nc.dram_tensor("o", [R, D], mybir.dt.float32, kind="ExternalOutput")

    with tile.TileContext(nc) as tc:
        with contextlib.ExitStack() as ctx:
            const = ctx.enter_context(tc.tile_pool(name="const", bufs=1))
            gp = ctx.enter_context(tc.tile_pool(name="gp", bufs=6))
            mp = ctx.enter_context(tc.tile_pool(name="mp", bufs=6))
            sp = ctx.enter_context(tc.tile_pool(name="sp", bufs=3))
            hp = ctx.enter_context(tc.tile_pool(name="hp", bufs=3))
            pa = ctx.enter_context(tc.tile_pool(name="pa", bufs=2, space="PSUM"))
            pz = ctx.enter_context(tc.tile_pool(name="pz", bufs=2, space="PSUM"))
            po = ctx.enter_context(tc.tile_pool(name="po", bufs=2, space="PSUM"))

            colsSB = const.tile([P, S], mybir.dt.int32)
            rlSB = const.tile([P, S], mybir.dt.float32)
            nc.sync.dma_start(out=colsSB[:], in_=cols_d[:])
            nc.sync.dma_start(out=rlSB[:], in_=rl_d[:])
            w1SB = const.tile([D, D], mybir.dt.float32)
            b1SB = const.tile([P, 1], mybir.dt.float32)
            wpSB = const.tile([D, D], mybir.dt.float32)
            bpSB = const.tile([1, D], mybir.dt.float32)
            nc.sync.dma_start(out=w1SB[:], in_=w1_d[:])
            nc.sync.dma_start(out=b1SB[:], in_=b1_d[:])
            nc.sync.dma_start(out=wpSB[:], in_=wp_d[:])
            nc.sync.dma_start(out=bpSB[:], in_=bp_d[:])
            ones1 = const.tile([1, P], mybir.dt.float32)
            nc.vector.memset(ones1[:], 1.0)
            iotaI = const.tile([P, P], mybir.dt.int32)
            nc.gpsimd.iota(iotaI[:], pattern=[[1, P]], base=0,
                           channel_multiplier=0)
            iotaF = const.tile([P, P], mybir.dt.float32)
            nc.vector.tensor_copy(iotaF[:], iotaI[:])

            for b in range(NB):
                rows_b = min(P, R - b * P)
                T_b = int(T_arr[b])
                psumA = pa.tile([P, P], mybir.dt.float32, tag="pa")
                for j in range(T_b):
                    s = int(offs[b]) + j
                    gb = gp.tile([P, P], mybir.dt.float32, tag="g")
                    nc.gpsimd.indirect_dma_start(
                        out=gb[:], out_offset=None, in_=hf_d[:],
                        in_offset=bass.IndirectOffsetOnAxis(
                            ap=colsSB[:, s:s + 1], axis=0))
                    M = mp.tile([P, P], mybir.dt.float32, tag="m")
                    nc.vector.tensor_scalar(
                        out=M[:], in0=iotaF[:], scalar1=rlSB[:, s:s + 1],
                        scalar2=None, op0=mybir.AluOpType.is_equal)
                    nc.tensor.matmul(psumA[:], lhsT=gb[:], rhs=M[:],
                                     start=(j == 0), stop=(j == T_b - 1))
                sA1 = sp.tile([P, P], mybir.dt.float32, tag="sa")
                nc.vector.tensor_copy(sA1[:], psumA[:])       # agg1T [feat, rows]
                psumZ = pz.tile([P, P], mybir.dt.float32, tag="pz")
                nc.tensor.matmul(psumZ[:], lhsT=w1SB[:], rhs=sA1[:],
                                 start=True, stop=True)        # (agg1@W1).T
                t1 = hp.tile([P, P], mybir.dt.float32, tag="t1")
                nc.scalar.activation(t1[:], psumZ[:],
                                     mybir.ActivationFunctionType.Relu,
                                     bias=b1SB[:])              # relu(zT + b1)
                h2T = hp.tile([P, P], mybir.dt.float32, tag="h2")
                nc.vector.tensor_add(h2T[:], t1[:], sA1[:])     # + agg1 (residual)
                psumO = po.tile([P, P], mybir.dt.float32, tag="po")
                nc.tensor.matmul(psumO[:], lhsT=h2T[:], rhs=wpSB[:],
                                 start=True, stop=False)
                nc.tensor.matmul(psumO[:], lhsT=ones1[:], rhs=bpSB[:],
                                 start=False, stop=True)        # h2@Wp + bp
                osb = hp.tile([P, P], mybir.dt.float32, tag="o")
                nc.vector.tensor_copy(osb[:], psumO[:])
                nc.sync.dma_start(out=o_d[b * P:b * P + rows_b, :],
                                  in_=osb[:rows_b, :])
    nc.compile()
    return nc


def _run(nc, in_maps):
    global LAST_EXEC_NS
    res = run_bass_kernel_spmd(nc, in_maps, core_ids=list(range(NC)),
                               trace=PROFILE)
    if PROFILE:
        LAST_EXEC_NS.append(res.exec_time_ns)
    return res.results


def kernel(x, edge_index, W0, b0, W1, b1, Wp, bp):
    global LAST_EXEC_NS
    LAST_EXEC_NS = []
    if PROFILE:
        _install_ntff_shim()
    x = np.ascontiguousarray(x, dtype=np.float32)
    W0 = np.ascontiguousarray(W0, dtype=np.float32)
    W1 = np.ascontiguousarray(W1, dtype=np.float32)
    Wp = np.ascontiguousarray(Wp, dtype=np.float32)
    colsT, rlT, T_arr, offs = _prep_edges(np.asarray(edge_index))

    nc0 = _build_layer0(T_arr, offs)
    in0 = [{"x": x, "cols": colsT[k], "rl": rlT[k],
            "w0": W0, "b0": np.asarray(b0, np.float32).reshape(1, D)}
           for k in range(NC)]
    res0 = _run(nc0, in0)
    hfull = np.concatenate([res0[k]["h"] for k in range(NC)], axis=0)

    nc1 = _build_layer1(T_arr, offs)
    in1 = [{"hf": hfull, "cols": colsT[k], "rl": rlT[k],
            "w1": W1, "b1": np.asarray(b1, np.float32).reshape(P, 1),
            "wp": Wp, "bp": np.asarray(bp, np.float32).reshape(1, D)}
           for k in range(NC)]
    res1 = _run(nc1, in1)
    out = np.concatenate([res1[k]["o"] for k in range(NC)], axis=0)
    return out

